# revision 32
# baseline (speedup 1.0000x reference)
"""AttentionDecoder (topk_masking) Trainium2 kernel.

Algorithm (matches the jax reference):
  combined = cat([current, context])           # [1, 2E]
  query    = combined @ Wq.T                   # [1, H]
  scores   = query @ Wk @ cand.T               # [1, N]  (keys folded into w)
  masked softmax -> top-50 filter -> renormalize -> log probs + categorical
  sample (jax key 42).

Distribution: candidates are sharded row-wise over 8 NeuronCores (25000
rows/core, zero-padded to 25088 = 196*128) and shipped pre-transposed,
compressed to bf16 (2 bytes/element — half the HBM traffic of fp32).
The mask is folded into the data: masked candidate columns are replaced
host-side by -1e9 * w / |w|^2, so their score is ~-1e9 and exp
underflows to exact 0 — no separate mask plane or masking arithmetic on
device.  Per 128-candidate slice two accumulating PE matmuls (candidate
tile stationary, rhs = w_hi then w_lo, the bf16 split of the query
vector) produce the scores in a single fp32 PSUM column; DVE just
copies PSUM->SBUF, and ACT computes the partial softmax statistics
sum(exp(s - shift)) (fixed shift) per chunk directly into the output
tile, all overlapped with the next chunk's DMA+matmuls.  Outputs stream
out on the second (ACT) HWDGE ring while the input stream still owns
the first.

The bf16 rounding is fully healed on the host: it gathers the per-core
scores + softmax partials, merges the statistics (the "all-reduce"
step), selects the top-100 by device score — the true top-50 is inside
with a containment slack 25x the worst score error — and re-scores
exactly those 100 candidates on the reference fp32 path.  The softmax
normalizer cancels in the renormalized top-50 distribution, so the
outputs are fp32-exact.

The kernel is written in raw Bass (hand-placed semaphores, no Tile
scheduler) so the only fixed overhead is the NRT preamble — the Tile
exit-barrier butterfly (~10 us) is avoided.  The w vector rides in the
first bytes of the big candidate tensor so the PE can start as soon as
chunk 0 lands.

The kernel is memory-bound on the HBM read of the 51.2 MB of compressed
candidate embeddings (6.4 MB/core, HBM shared per core pair).
"""

import os

import numpy as np

E = 128
N_TOTAL = 200000
NCORES = 8
SHARD = N_TOTAL // NCORES       # 25000 rows per core
NSLICE = 196                    # 128-wide score slices per core
PAD = NSLICE * 128              # 25088 padded rows per core
NCHUNK = 4
SPCS = [56, 56, 56, 28]         # slices per chunk (small last chunk = short tail)
SOFF = [0, 56, 112, 168]        # slice offset of each chunk
CHBS = [s * 256 for s in SPCS]  # bytes per chunk (bf16: 128 cols * 2B per slice)
PREFIX = 4                      # [wh, wl] bf16
TOTB = PREFIX + 2 * PAD         # 50180 bytes per partition
MASKSCORE = np.float32(-1.0e9)  # masked columns score ~ -1e9 (exp -> 0)
TOPK = 50
RESCORE = 100                   # candidates re-scored exactly on host

_NC_CACHE = {}
LAST_RESULTS = None  # BassKernelResults of the most recent run (for profiling)
LAST_SCORES = None  # gathered masked scores of the most recent run (diagnostics)


def _build_nc():
    """Raw-Bass per-core program (identical on all 8 cores)."""
    import concourse.bacc as bacc
    from concourse import mybir

    f32 = mybir.dt.float32
    bf16 = mybir.dt.bfloat16
    u8 = mybir.dt.uint8
    Exp = mybir.ActivationFunctionType.Exp

    nc = bacc.Bacc(
        "TRN2",
        target_bir_lowering=False,
        debug=False,
        enable_asserts=False,
        num_devices=NCORES,
    )

    candB = nc.dram_tensor("candB", [128, TOTB], u8, kind="ExternalInput")
    negshift = nc.dram_tensor("negshift", [128, 1], f32, kind="ExternalInput")
    # outbuf[:, :196] = masked scores, [:, 196:200] = per-chunk per-partition
    # sum(exp(s - shift)) partials (host sums them)
    outb = nc.dram_tensor("outbuf", [128, NSLICE + NCHUNK], f32, kind="ExternalOutput")

    from contextlib import ExitStack

    with ExitStack() as ctx:
        ec = ctx.enter_context
        c0 = ec(nc.sbuf_tensor("c0", [128, PREFIX + CHBS[0]], u8))
        cbufs = [c0] + [
            ec(nc.sbuf_tensor(f"c{i}", [128, CHBS[i]], u8))
            for i in range(1, NCHUNK)
        ]
        ns = ec(nc.sbuf_tensor("ns", [128, 1], f32))
        outsb = ec(nc.sbuf_tensor("outsb", [128, NSLICE + NCHUNK], f32))
        ets = [
            ec(nc.sbuf_tensor(f"et{i}", [128, SPCS[i]], f32))
            for i in range(NCHUNK)
        ]
        psH = [
            ec(nc.psum_tensor("psHA", [128, 512], f32)),
            ec(nc.psum_tensor("psHB", [128, 512], f32)),
        ]
        ch_sems = [ec(nc.semaphore(f"ch_sem{c}")) for c in range(NCHUNK)]
        out_sem = ec(nc.semaphore("out_sem"))
        out2_sem = ec(nc.semaphore("out2_sem"))
        ns_sem = ec(nc.semaphore("ns_sem"))
        pe_sem = ec(nc.semaphore("pe_sem"))
        dve_sem = ec(nc.semaphore("dve_sem"))
        act_sem = ec(nc.semaphore("act_sem"))

        def chunk_hi(c):
            t = cbufs[c]
            base = PREFIX if c == 0 else 0
            return t[:, base : base + CHBS[c]].bitcast(bf16)

        with nc.Block() as block:

            @block.sync
            def _(sync):
                off = 0
                for c in range(NCHUNK):
                    pre = PREFIX if c == 0 else 0
                    sync.dma_start(
                        cbufs[c][:], candB[:, off : off + pre + CHBS[c]]
                    ).then_inc(ch_sems[c], 16)
                    off += pre + CHBS[c]
                sync.wait_ge(out_sem, 16)
                sync.wait_ge(out2_sem, 16)

            @block.scalar
            def _(scalar):
                scalar.dma_start(ns[:], negshift[:]).then_inc(ns_sem, 16)
                scalar.wait_ge(ns_sem, 16)
                for c in range(NCHUNK):
                    scalar.wait_ge(dve_sem, c + 1)
                    scalar.activation(
                        ets[c][:],
                        outsb[:, SOFF[c] : SOFF[c] + SPCS[c]],
                        Exp,
                        bias=ns[:],
                        scale=1.0,
                        accum_out=outsb[:, NSLICE + c : NSLICE + c + 1],
                    ).then_inc(act_sem)
                    if c == 2:
                        # chunks 0-2 scores are final: stream them out on
                        # this (otherwise idle) HWDGE ring, hidden under
                        # the tail of the input stream
                        scalar.dma_start(
                            outb[:, 0 : SOFF[3]], outsb[:, 0 : SOFF[3]]
                        ).then_inc(out_sem, 16)
                scalar.wait_ge(act_sem, NCHUNK)
                scalar.dma_start(
                    outb[:, SOFF[3] :], outsb[:, SOFF[3] :]
                ).then_inc(out2_sem, 16)

            @block.tensor
            def _(tensor):
                w_bf = c0[:, 0:4].bitcast(bf16)  # [128, 2] = [wh, wl]
                for c in range(NCHUNK):
                    tensor.wait_ge(ch_sems[c], 16)
                    if c >= 2:
                        tensor.wait_ge(dve_sem, c - 1)  # psum bank reuse guard
                    pH = psH[c % 2]
                    hi = chunk_hi(c)
                    for si in range(SPCS[c]):
                        nc.tensor.matmul(
                            pH[:, si : si + 1],
                            hi[:, si * 128 : (si + 1) * 128],
                            w_bf[:, 0:1],
                            start=True,
                            stop=False,
                        )
                        mm = nc.tensor.matmul(
                            pH[:, si : si + 1],
                            hi[:, si * 128 : (si + 1) * 128],
                            w_bf[:, 1:2],
                            start=False,
                            stop=True,
                        )
                    mm.then_inc(pe_sem)

            @block.vector
            def _(vector):
                for c in range(NCHUNK):
                    vector.wait_ge(pe_sem, c + 1)
                    vector.tensor_copy(
                        outsb[:, SOFF[c] : SOFF[c] + SPCS[c]],
                        psH[c % 2][:, 0 : SPCS[c]],
                    ).then_inc(dve_sem)
                    vector.drain()

    nc.finalize()
    return nc


def get_nc():
    if "nc" not in _NC_CACHE:
        _NC_CACHE["nc"] = _build_nc()
    return _NC_CACHE["nc"]


def make_in_maps(cand, w, mask_np, shift):
    """Shard + lay out host inputs for the 8 cores."""
    import ml_dtypes

    bf16 = ml_dtypes.bfloat16

    wf = w.reshape(E)
    wh = wf.astype(bf16)
    wl = (wf - wh.astype(np.float32)).astype(bf16)
    w_bf = np.stack([wh, wl], axis=1)  # [128, 2] bf16
    prefix = np.ascontiguousarray(w_bf).view(np.uint8)  # [128, 4]

    # masked/padded candidate columns become this vector: score ~ -1e9
    mvec = (float(MASKSCORE) / float(wf @ wf)) * wf  # [128] f32

    negshift = np.full((128, 1), -shift, np.float32)

    candT = cand.T  # [128, N] view
    in_maps = []
    for c in range(NCORES):
        xc = np.empty((128, PAD), np.float32)
        xc[:, :SHARD] = candT[:, c * SHARD : (c + 1) * SHARD]
        mflat = np.zeros(PAD, bool)
        mflat[:SHARD] = mask_np[0, c * SHARD : (c + 1) * SHARD] != 0
        xc[:, ~mflat] = mvec[:, None]
        hi_u8 = xc.astype(bf16).view(np.uint8)  # [128, 2*PAD]
        candB = np.empty((128, TOTB), np.uint8)
        candB[:, 0:PREFIX] = prefix
        candB[:, PREFIX:] = hi_u8
        in_maps.append({"candB": candB, "negshift": negshift})
    return in_maps


def _run_spmd(nc, in_maps):
    """run_bass_kernel_spmd with the optional NTFF-trace path made safe.

    If BASS_TRACE is set in the environment, run_bass_kernel_spmd needs the
    axon NTFF hook (antenv.axon_hooks) and an artifact upload; neither is
    guaranteed on this image.  Register the hook from the boot shim when
    missing, keep artifact upload local, and fall back to an untraced run
    on any trace-infrastructure failure.
    """
    import sys
    import types

    import concourse.bass_utils as bu

    try:
        import antenv.axon_hooks  # noqa: F401
    except ImportError:
        try:
            from trn_agent_boot.trn_boot import _ntff_profile_via_ctypes

            hook = _ntff_profile_via_ctypes("/opt/axon/libaxon_pjrt.so")
            mod = types.ModuleType("antenv.axon_hooks")
            mod.get_axon_ntff_profile_hook = lambda: hook
            sys.modules["antenv.axon_hooks"] = mod
        except Exception:
            os.environ["BASS_NEVER_TRACE"] = "1"

    if not getattr(bu.upload_artifacts, "_safe", False):
        orig_upload = bu.upload_artifacts

        def _safe_upload(tmpdir):
            try:
                return orig_upload(tmpdir)
            except Exception:
                return tmpdir

        _safe_upload._safe = True
        bu.upload_artifacts = _safe_upload

    try:
        return bu.run_bass_kernel_spmd(nc, in_maps, list(range(NCORES)))
    except Exception:
        if os.environ.get("BASS_NEVER_TRACE"):
            raise
        os.environ["BASS_NEVER_TRACE"] = "1"
        return bu.run_bass_kernel_spmd(nc, in_maps, list(range(NCORES)))


def kernel(current_node_emb, context_emb, candidate_node_embs, Wq, Wk, mask):
    global LAST_RESULTS, LAST_SCORES

    cur = np.asarray(current_node_emb, np.float32)
    ctxe = np.asarray(context_emb, np.float32)
    cand = np.ascontiguousarray(np.asarray(candidate_node_embs, np.float32))
    Wq_np = np.asarray(Wq, np.float32)
    Wk_np = np.asarray(Wk, np.float32)
    mask_np = np.asarray(mask)

    # tiny query projection; scores = w @ cand.T with w = (combined @ Wq.T) @ Wk
    combined = np.concatenate([cur, ctxe], axis=1)  # [1, 2E]
    query = (combined @ Wq_np.T).astype(np.float32)  # [1, H]
    w = (query @ Wk_np).astype(np.float32)  # [1, E]

    # fixed exp shift: safe upper bound on any score
    shift = float(max(40.0, 16.0 * np.linalg.norm(w)))

    in_maps = make_in_maps(cand, w, mask_np, shift)
    nc = get_nc()
    res = _run_spmd(nc, in_maps)
    LAST_RESULTS = res

    # ---- gather / merge ----
    all_scores = np.empty(N_TOTAL, np.float32)
    rowsums = np.empty((NCORES, 128), np.float64)
    for c in range(NCORES):
        ob = np.asarray(res.results[c]["outbuf"])  # [128, 200]
        all_scores[c * SHARD : (c + 1) * SHARD] = ob[:, :NSLICE].T.reshape(-1)[:SHARD]
        rowsums[c] = ob[:, NSLICE:].astype(np.float64).sum(axis=1)
    LAST_SCORES = all_scores

    # top-RESCORE candidates by device score; re-score them exactly on the
    # reference fp32 path (keys = cand @ Wk.T, s = query @ keys.T)
    sel = np.argpartition(all_scores, N_TOTAL - RESCORE)[N_TOTAL - RESCORE :]
    keys_sel = (cand[sel] @ Wk_np.T).astype(np.float32)  # [R, H]
    s_sel = (query @ keys_sel.T).astype(np.float32)[0]  # [R]

    # merge softmax statistics (the "all-reduce" step, done at gather time)
    m = np.float32(s_sel.max())
    Z = np.float32(np.exp(np.float64(shift) - np.float64(m)) * rowsums.sum())

    # exact probabilities of the re-scored candidates; top-50 threshold in
    # probability space, exactly like the reference
    p_sel = (np.exp(s_sel - m) / Z).astype(np.float32)
    th = np.sort(p_sel)[-TOPK]
    keep = p_sel >= th
    p_top = p_sel * keep
    S = p_top.sum(dtype=np.float32)
    fil_top = (p_top / (S + np.float32(1e-10))).astype(np.float32)

    log_probs_all = np.full((1, N_TOTAL), np.log(np.float32(1e-10)), np.float32)
    logits = np.full((1, N_TOTAL), -np.inf, np.float32)
    sel_keep = sel[keep]
    fil_keep = fil_top[keep]
    log_probs_all[0, sel_keep] = np.log(fil_keep + np.float32(1e-10))
    logits[0, sel_keep] = np.log(fil_keep)

    # categorical sample with jax key 42 (on host CPU, exact reference RNG)
    import jax

    cpu = jax.devices("cpu")[0]
    with jax.default_device(cpu):
        action_idx = np.asarray(
            jax.random.categorical(
                jax.random.key(42), jax.numpy.asarray(logits), axis=1
            )
        )
    log_prob_action = np.take_along_axis(logits, action_idx[:, None], axis=1)[:, 0]

    return log_probs_all, log_prob_action, action_idx


# revision 33
# speedup vs baseline: 1.0049x; 1.0049x over previous
"""AttentionDecoder (topk_masking) Trainium2 kernel.

Algorithm (matches the jax reference):
  combined = cat([current, context])           # [1, 2E]
  query    = combined @ Wq.T                   # [1, H]
  scores   = query @ Wk @ cand.T               # [1, N]  (keys folded into w)
  masked softmax -> top-50 filter -> renormalize -> log probs + categorical
  sample (jax key 42).

Distribution: candidates are sharded row-wise over 8 NeuronCores (25000
rows/core, zero-padded to 25088 = 196*128) and shipped pre-transposed,
compressed to bf16 (2 bytes/element — half the HBM traffic of fp32).
The mask is folded into the data: masked candidate columns are replaced
host-side by -1e9 * w / |w|^2, so their score is ~-1e9 and exp
underflows to exact 0 — no separate mask plane or masking arithmetic on
device.  Per 128-candidate slice two accumulating PE matmuls (candidate
tile stationary, rhs = w_hi then w_lo, the bf16 split of the query
vector) produce the scores in a single fp32 PSUM column; DVE just
copies PSUM->SBUF, and ACT computes the partial softmax statistics
sum(exp(s - shift)) (fixed shift) per chunk directly into the output
tile, all overlapped with the next chunk's DMA+matmuls.  Outputs stream
out on the second (ACT) HWDGE ring while the input stream still owns
the first.

The bf16 rounding is fully healed on the host: it gathers the per-core
scores + softmax partials, merges the statistics (the "all-reduce"
step), selects the top-100 by device score — the true top-50 is inside
with a containment slack 25x the worst score error — and re-scores
exactly those 100 candidates on the reference fp32 path.  The softmax
normalizer cancels in the renormalized top-50 distribution, so the
outputs are fp32-exact.

The kernel is written in raw Bass (hand-placed semaphores, no Tile
scheduler) so the only fixed overhead is the NRT preamble — the Tile
exit-barrier butterfly (~10 us) is avoided.  The w vector rides in the
first bytes of the big candidate tensor so the PE can start as soon as
chunk 0 lands.

The kernel is memory-bound on the HBM read of the 51.2 MB of compressed
candidate embeddings (6.4 MB/core, HBM shared per core pair).
"""

import os

import numpy as np

E = 128
N_TOTAL = 200000
NCORES = 8
SHARD = N_TOTAL // NCORES       # 25000 rows per core
NSLICE = 196                    # 128-wide score slices per core
PAD = NSLICE * 128              # 25088 padded rows per core
NCHUNK = 4
SPCS = [56, 56, 56, 28]         # slices per chunk (small last chunk = short tail)
SOFF = [0, 56, 112, 168]        # slice offset of each chunk
CHBS = [s * 256 for s in SPCS]  # bytes per chunk (bf16: 128 cols * 2B per slice)
PREFIX = 4                      # [wh, wl] bf16
TOTB = PREFIX + 2 * PAD         # 50180 bytes per partition
MASKSCORE = np.float32(-1.0e9)  # masked columns score ~ -1e9 (exp -> 0)
TOPK = 50
RESCORE = 100                   # candidates re-scored exactly on host

_NC_CACHE = {}
LAST_RESULTS = None  # BassKernelResults of the most recent run (for profiling)
LAST_SCORES = None  # gathered masked scores of the most recent run (diagnostics)


def _build_nc():
    """Raw-Bass per-core program (identical on all 8 cores)."""
    import concourse.bacc as bacc
    from concourse import mybir

    f32 = mybir.dt.float32
    bf16 = mybir.dt.bfloat16
    u8 = mybir.dt.uint8
    Exp = mybir.ActivationFunctionType.Exp

    nc = bacc.Bacc(
        "TRN2",
        target_bir_lowering=False,
        debug=False,
        enable_asserts=False,
        num_devices=NCORES,
    )

    candB = nc.dram_tensor("candB", [128, TOTB], u8, kind="ExternalInput")
    negshift = nc.dram_tensor("negshift", [128, 1], f32, kind="ExternalInput")
    # outbuf[:, :196] = masked scores, [:, 196:200] = per-chunk per-partition
    # sum(exp(s - shift)) partials (host sums them)
    outb = nc.dram_tensor("outbuf", [128, NSLICE + NCHUNK], f32, kind="ExternalOutput")

    from contextlib import ExitStack

    with ExitStack() as ctx:
        ec = ctx.enter_context
        c0 = ec(nc.sbuf_tensor("c0", [128, PREFIX + CHBS[0]], u8))
        cbufs = [c0] + [
            ec(nc.sbuf_tensor(f"c{i}", [128, CHBS[i]], u8))
            for i in range(1, NCHUNK)
        ]
        ns = ec(nc.sbuf_tensor("ns", [128, 1], f32))
        outsb = ec(nc.sbuf_tensor("outsb", [128, NSLICE + NCHUNK], f32))
        ets = [
            ec(nc.sbuf_tensor(f"et{i}", [128, SPCS[i]], f32))
            for i in range(NCHUNK)
        ]
        psH = [
            ec(nc.psum_tensor("psHA", [128, 512], f32)),
            ec(nc.psum_tensor("psHB", [128, 512], f32)),
        ]
        ch_sems = [ec(nc.semaphore(f"ch_sem{c}")) for c in range(NCHUNK)]
        out_sem = ec(nc.semaphore("out_sem"))
        out2_sem = ec(nc.semaphore("out2_sem"))
        ns_sem = ec(nc.semaphore("ns_sem"))
        pe_sem = ec(nc.semaphore("pe_sem"))
        dve_sem = ec(nc.semaphore("dve_sem"))
        act_sem = ec(nc.semaphore("act_sem"))

        def chunk_hi(c):
            t = cbufs[c]
            base = PREFIX if c == 0 else 0
            return t[:, base : base + CHBS[c]].bitcast(bf16)

        with nc.Block() as block:

            @block.sync
            def _(sync):
                off = 0
                for c in range(NCHUNK):
                    pre = PREFIX if c == 0 else 0
                    sync.dma_start(
                        cbufs[c][:], candB[:, off : off + pre + CHBS[c]]
                    ).then_inc(ch_sems[c], 16)
                    off += pre + CHBS[c]
                sync.wait_ge(out_sem, 16)
                sync.wait_ge(out2_sem, 16)

            @block.scalar
            def _(scalar):
                scalar.dma_start(ns[:], negshift[:]).then_inc(ns_sem, 16)
                scalar.wait_ge(ns_sem, 16)
                for c in range(NCHUNK):
                    scalar.wait_ge(dve_sem, c + 1)
                    scalar.activation(
                        ets[c][:],
                        outsb[:, SOFF[c] : SOFF[c] + SPCS[c]],
                        Exp,
                        bias=ns[:],
                        scale=1.0,
                        accum_out=outsb[:, NSLICE + c : NSLICE + c + 1],
                    ).then_inc(act_sem)
                    if c == 2:
                        # chunks 0-2 scores are final: stream them out on
                        # this (otherwise idle) HWDGE ring — but only once
                        # the input stream is done, so the out transfer
                        # doesn't steal input bandwidth mid-stream
                        scalar.wait_ge(ch_sems[NCHUNK - 1], 16)
                        scalar.dma_start(
                            outb[:, 0 : SOFF[3]], outsb[:, 0 : SOFF[3]]
                        ).then_inc(out_sem, 16)
                scalar.wait_ge(act_sem, NCHUNK)
                scalar.dma_start(
                    outb[:, SOFF[3] :], outsb[:, SOFF[3] :]
                ).then_inc(out2_sem, 16)

            @block.tensor
            def _(tensor):
                w_bf = c0[:, 0:4].bitcast(bf16)  # [128, 2] = [wh, wl]
                for c in range(NCHUNK):
                    tensor.wait_ge(ch_sems[c], 16)
                    if c >= 2:
                        tensor.wait_ge(dve_sem, c - 1)  # psum bank reuse guard
                    pH = psH[c % 2]
                    hi = chunk_hi(c)
                    for si in range(SPCS[c]):
                        nc.tensor.matmul(
                            pH[:, si : si + 1],
                            hi[:, si * 128 : (si + 1) * 128],
                            w_bf[:, 0:1],
                            start=True,
                            stop=False,
                        )
                        mm = nc.tensor.matmul(
                            pH[:, si : si + 1],
                            hi[:, si * 128 : (si + 1) * 128],
                            w_bf[:, 1:2],
                            start=False,
                            stop=True,
                        )
                    mm.then_inc(pe_sem)

            @block.vector
            def _(vector):
                for c in range(NCHUNK):
                    vector.wait_ge(pe_sem, c + 1)
                    vector.tensor_copy(
                        outsb[:, SOFF[c] : SOFF[c] + SPCS[c]],
                        psH[c % 2][:, 0 : SPCS[c]],
                    ).then_inc(dve_sem)
                    vector.drain()

    nc.finalize()
    return nc


def get_nc():
    if "nc" not in _NC_CACHE:
        _NC_CACHE["nc"] = _build_nc()
    return _NC_CACHE["nc"]


def make_in_maps(cand, w, mask_np, shift):
    """Shard + lay out host inputs for the 8 cores."""
    import ml_dtypes

    bf16 = ml_dtypes.bfloat16

    wf = w.reshape(E)
    wh = wf.astype(bf16)
    wl = (wf - wh.astype(np.float32)).astype(bf16)
    w_bf = np.stack([wh, wl], axis=1)  # [128, 2] bf16
    prefix = np.ascontiguousarray(w_bf).view(np.uint8)  # [128, 4]

    # masked/padded candidate columns become this vector: score ~ -1e9
    mvec = (float(MASKSCORE) / float(wf @ wf)) * wf  # [128] f32

    negshift = np.full((128, 1), -shift, np.float32)

    candT = cand.T  # [128, N] view
    in_maps = []
    for c in range(NCORES):
        xc = np.empty((128, PAD), np.float32)
        xc[:, :SHARD] = candT[:, c * SHARD : (c + 1) * SHARD]
        mflat = np.zeros(PAD, bool)
        mflat[:SHARD] = mask_np[0, c * SHARD : (c + 1) * SHARD] != 0
        xc[:, ~mflat] = mvec[:, None]
        hi_u8 = xc.astype(bf16).view(np.uint8)  # [128, 2*PAD]
        candB = np.empty((128, TOTB), np.uint8)
        candB[:, 0:PREFIX] = prefix
        candB[:, PREFIX:] = hi_u8
        in_maps.append({"candB": candB, "negshift": negshift})
    return in_maps


def _run_spmd(nc, in_maps):
    """run_bass_kernel_spmd with the optional NTFF-trace path made safe.

    If BASS_TRACE is set in the environment, run_bass_kernel_spmd needs the
    axon NTFF hook (antenv.axon_hooks) and an artifact upload; neither is
    guaranteed on this image.  Register the hook from the boot shim when
    missing, keep artifact upload local, and fall back to an untraced run
    on any trace-infrastructure failure.
    """
    import sys
    import types

    import concourse.bass_utils as bu

    try:
        import antenv.axon_hooks  # noqa: F401
    except ImportError:
        try:
            from trn_agent_boot.trn_boot import _ntff_profile_via_ctypes

            hook = _ntff_profile_via_ctypes("/opt/axon/libaxon_pjrt.so")
            mod = types.ModuleType("antenv.axon_hooks")
            mod.get_axon_ntff_profile_hook = lambda: hook
            sys.modules["antenv.axon_hooks"] = mod
        except Exception:
            os.environ["BASS_NEVER_TRACE"] = "1"

    if not getattr(bu.upload_artifacts, "_safe", False):
        orig_upload = bu.upload_artifacts

        def _safe_upload(tmpdir):
            try:
                return orig_upload(tmpdir)
            except Exception:
                return tmpdir

        _safe_upload._safe = True
        bu.upload_artifacts = _safe_upload

    try:
        return bu.run_bass_kernel_spmd(nc, in_maps, list(range(NCORES)))
    except Exception:
        if os.environ.get("BASS_NEVER_TRACE"):
            raise
        os.environ["BASS_NEVER_TRACE"] = "1"
        return bu.run_bass_kernel_spmd(nc, in_maps, list(range(NCORES)))


def kernel(current_node_emb, context_emb, candidate_node_embs, Wq, Wk, mask):
    global LAST_RESULTS, LAST_SCORES

    cur = np.asarray(current_node_emb, np.float32)
    ctxe = np.asarray(context_emb, np.float32)
    cand = np.ascontiguousarray(np.asarray(candidate_node_embs, np.float32))
    Wq_np = np.asarray(Wq, np.float32)
    Wk_np = np.asarray(Wk, np.float32)
    mask_np = np.asarray(mask)

    # tiny query projection; scores = w @ cand.T with w = (combined @ Wq.T) @ Wk
    combined = np.concatenate([cur, ctxe], axis=1)  # [1, 2E]
    query = (combined @ Wq_np.T).astype(np.float32)  # [1, H]
    w = (query @ Wk_np).astype(np.float32)  # [1, E]

    # fixed exp shift: safe upper bound on any score
    shift = float(max(40.0, 16.0 * np.linalg.norm(w)))

    in_maps = make_in_maps(cand, w, mask_np, shift)
    nc = get_nc()
    res = _run_spmd(nc, in_maps)
    LAST_RESULTS = res

    # ---- gather / merge ----
    all_scores = np.empty(N_TOTAL, np.float32)
    rowsums = np.empty((NCORES, 128), np.float64)
    for c in range(NCORES):
        ob = np.asarray(res.results[c]["outbuf"])  # [128, 200]
        all_scores[c * SHARD : (c + 1) * SHARD] = ob[:, :NSLICE].T.reshape(-1)[:SHARD]
        rowsums[c] = ob[:, NSLICE:].astype(np.float64).sum(axis=1)
    LAST_SCORES = all_scores

    # top-RESCORE candidates by device score; re-score them exactly on the
    # reference fp32 path (keys = cand @ Wk.T, s = query @ keys.T)
    sel = np.argpartition(all_scores, N_TOTAL - RESCORE)[N_TOTAL - RESCORE :]
    keys_sel = (cand[sel] @ Wk_np.T).astype(np.float32)  # [R, H]
    s_sel = (query @ keys_sel.T).astype(np.float32)[0]  # [R]

    # merge softmax statistics (the "all-reduce" step, done at gather time)
    m = np.float32(s_sel.max())
    Z = np.float32(np.exp(np.float64(shift) - np.float64(m)) * rowsums.sum())

    # exact probabilities of the re-scored candidates; top-50 threshold in
    # probability space, exactly like the reference
    p_sel = (np.exp(s_sel - m) / Z).astype(np.float32)
    th = np.sort(p_sel)[-TOPK]
    keep = p_sel >= th
    p_top = p_sel * keep
    S = p_top.sum(dtype=np.float32)
    fil_top = (p_top / (S + np.float32(1e-10))).astype(np.float32)

    log_probs_all = np.full((1, N_TOTAL), np.log(np.float32(1e-10)), np.float32)
    logits = np.full((1, N_TOTAL), -np.inf, np.float32)
    sel_keep = sel[keep]
    fil_keep = fil_top[keep]
    log_probs_all[0, sel_keep] = np.log(fil_keep + np.float32(1e-10))
    logits[0, sel_keep] = np.log(fil_keep)

    # categorical sample with jax key 42 (on host CPU, exact reference RNG)
    import jax

    cpu = jax.devices("cpu")[0]
    with jax.default_device(cpu):
        action_idx = np.asarray(
            jax.random.categorical(
                jax.random.key(42), jax.numpy.asarray(logits), axis=1
            )
        )
    log_prob_action = np.take_along_axis(logits, action_idx[:, None], axis=1)[:, 0]

    return log_probs_all, log_prob_action, action_idx


# revision 34
# speedup vs baseline: 1.1929x; 1.1872x over previous
"""AttentionDecoder (topk_masking) Trainium2 kernel.

Algorithm (matches the jax reference):
  combined = cat([current, context])           # [1, 2E]
  query    = combined @ Wq.T                   # [1, H]
  scores   = query @ Wk @ cand.T               # [1, N]  (keys folded into w)
  masked softmax -> top-50 filter -> renormalize -> log probs + categorical
  sample (jax key 42).

Distribution: candidates are sharded row-wise over 8 NeuronCores (25000
rows/core, zero-padded to 25088 = 196*128) and shipped pre-transposed,
compressed to bf16 (2 bytes/element — half the HBM traffic of fp32).
The mask is folded into the data: masked candidate columns are replaced
host-side by -1e9 * w / |w|^2, so their score is ~-1e9 and exp
underflows to exact 0 — no separate mask plane or masking arithmetic on
device.  Per 128-candidate slice two accumulating PE matmuls (candidate
tile stationary, rhs = w_hi then w_lo, the bf16 split of the query
vector) produce the scores in a single fp32 PSUM column; DVE just
copies PSUM->SBUF, and ACT computes the partial softmax statistics
sum(exp(s - shift)) (fixed shift) per chunk directly into the output
tile, all overlapped with the next chunk's DMA+matmuls.  Outputs stream
out on the second (ACT) HWDGE ring while the input stream still owns
the first.

The bf16 rounding is fully healed on the host: it gathers the per-core
scores + softmax partials, merges the statistics (the "all-reduce"
step), selects the top-100 by device score — the true top-50 is inside
with a containment slack 25x the worst score error — and re-scores
exactly those 100 candidates on the reference fp32 path.  The softmax
normalizer cancels in the renormalized top-50 distribution, so the
outputs are fp32-exact.

The kernel is written in raw Bass (hand-placed semaphores, no Tile
scheduler) so the only fixed overhead is the NRT preamble — the Tile
exit-barrier butterfly (~10 us) is avoided.  The w vector rides in the
first bytes of the big candidate tensor so the PE can start as soon as
chunk 0 lands.

The kernel is memory-bound on the HBM read of the 51.2 MB of compressed
candidate embeddings (6.4 MB/core, HBM shared per core pair).
"""

import os

import numpy as np

E = 128
N_TOTAL = 200000
NCORES = 8
SHARD = N_TOTAL // NCORES       # 25000 rows per core
NSLICE = 196                    # 128-wide score slices per core
PAD = NSLICE * 128              # 25088 padded rows per core
NCHUNK = 4
SPCS = [56, 56, 56, 28]         # slices per chunk (small last chunk = short tail)
SOFF = [0, 56, 112, 168]        # slice offset of each chunk
CHBS = [s * 128 for s in SPCS]  # bytes per chunk (fp8: 128 cols * 1B per slice)
PREFIX = 4                      # [wh8, wl8] fp8 + 2B pad
TOTB = PREFIX + PAD             # 25092 bytes per partition
MASKSCORE = np.float32(-2000.0)  # masked columns score (exp -> 0, in fp8 range)
TOPK = 50
RESCORE = 500                   # candidates re-scored exactly on host

_NC_CACHE = {}
LAST_RESULTS = None  # BassKernelResults of the most recent run (for profiling)
LAST_SCORES = None  # gathered masked scores of the most recent run (diagnostics)


def _build_nc():
    """Raw-Bass per-core program (identical on all 8 cores)."""
    import concourse.bacc as bacc
    from concourse import mybir

    f32 = mybir.dt.float32
    f8 = mybir.dt.float8e4
    u8 = mybir.dt.uint8
    Exp = mybir.ActivationFunctionType.Exp

    nc = bacc.Bacc(
        "TRN2",
        target_bir_lowering=False,
        debug=False,
        enable_asserts=False,
        num_devices=NCORES,
    )

    candB = nc.dram_tensor("candB", [128, TOTB], u8, kind="ExternalInput")
    negshift = nc.dram_tensor("negshift", [128, 1], f32, kind="ExternalInput")
    # outbuf[:, :196] = masked scores, [:, 196:200] = per-chunk per-partition
    # sum(exp(s - shift)) partials (host sums them)
    outb = nc.dram_tensor("outbuf", [128, NSLICE + NCHUNK], f32, kind="ExternalOutput")

    from contextlib import ExitStack

    with ExitStack() as ctx:
        ec = ctx.enter_context
        c0 = ec(nc.sbuf_tensor("c0", [128, PREFIX + CHBS[0]], u8))
        cbufs = [c0] + [
            ec(nc.sbuf_tensor(f"c{i}", [128, CHBS[i]], u8))
            for i in range(1, NCHUNK)
        ]
        ns = ec(nc.sbuf_tensor("ns", [128, 1], f32))
        outsb = ec(nc.sbuf_tensor("outsb", [128, NSLICE + NCHUNK], f32))
        ets = [
            ec(nc.sbuf_tensor(f"et{i}", [128, SPCS[i]], f32))
            for i in range(NCHUNK)
        ]
        psH = [
            ec(nc.psum_tensor("psHA", [128, 512], f32)),
            ec(nc.psum_tensor("psHB", [128, 512], f32)),
        ]
        ch_sems = [ec(nc.semaphore(f"ch_sem{c}")) for c in range(NCHUNK)]
        out_sem = ec(nc.semaphore("out_sem"))
        out2_sem = ec(nc.semaphore("out2_sem"))
        ns_sem = ec(nc.semaphore("ns_sem"))
        pe_sem = ec(nc.semaphore("pe_sem"))
        dve_sem = ec(nc.semaphore("dve_sem"))
        act_sem = ec(nc.semaphore("act_sem"))

        def chunk_hi(c):
            t = cbufs[c]
            base = PREFIX if c == 0 else 0
            return t[:, base : base + CHBS[c]].bitcast(f8)

        with nc.Block() as block:

            @block.sync
            def _(sync):
                off = 0
                for c in range(NCHUNK):
                    pre = PREFIX if c == 0 else 0
                    sync.dma_start(
                        cbufs[c][:], candB[:, off : off + pre + CHBS[c]]
                    ).then_inc(ch_sems[c], 16)
                    off += pre + CHBS[c]
                sync.wait_ge(out_sem, 16)
                sync.wait_ge(out2_sem, 16)

            @block.scalar
            def _(scalar):
                scalar.dma_start(ns[:], negshift[:]).then_inc(ns_sem, 16)
                scalar.wait_ge(ns_sem, 16)
                for c in range(NCHUNK):
                    scalar.wait_ge(dve_sem, c + 1)
                    scalar.activation(
                        ets[c][:],
                        outsb[:, SOFF[c] : SOFF[c] + SPCS[c]],
                        Exp,
                        bias=ns[:],
                        scale=1.0,
                        accum_out=outsb[:, NSLICE + c : NSLICE + c + 1],
                    ).then_inc(act_sem)
                    if c == 2:
                        # chunks 0-2 scores are final: stream them out on
                        # this (otherwise idle) HWDGE ring — but only once
                        # the input stream is done, so the out transfer
                        # doesn't steal input bandwidth mid-stream
                        scalar.wait_ge(ch_sems[NCHUNK - 1], 16)
                        scalar.dma_start(
                            outb[:, 0 : SOFF[3]], outsb[:, 0 : SOFF[3]]
                        ).then_inc(out_sem, 16)
                scalar.wait_ge(act_sem, NCHUNK)
                scalar.dma_start(
                    outb[:, SOFF[3] :], outsb[:, SOFF[3] :]
                ).then_inc(out2_sem, 16)

            @block.tensor
            def _(tensor):
                w_bf = c0[:, 0:2].bitcast(f8)  # [128, 2] = [wh8, wl8]
                for c in range(NCHUNK):
                    tensor.wait_ge(ch_sems[c], 16)
                    if c >= 2:
                        tensor.wait_ge(dve_sem, c - 1)  # psum bank reuse guard
                    pH = psH[c % 2]
                    hi = chunk_hi(c)
                    for si in range(SPCS[c]):
                        nc.tensor.matmul(
                            pH[:, si : si + 1],
                            hi[:, si * 128 : (si + 1) * 128],
                            w_bf[:, 0:1],
                            start=True,
                            stop=False,
                        )
                        mm = nc.tensor.matmul(
                            pH[:, si : si + 1],
                            hi[:, si * 128 : (si + 1) * 128],
                            w_bf[:, 1:2],
                            start=False,
                            stop=True,
                        )
                    mm.then_inc(pe_sem)

            @block.vector
            def _(vector):
                for c in range(NCHUNK):
                    vector.wait_ge(pe_sem, c + 1)
                    vector.tensor_copy(
                        outsb[:, SOFF[c] : SOFF[c] + SPCS[c]],
                        psH[c % 2][:, 0 : SPCS[c]],
                    ).then_inc(dve_sem)
                    vector.drain()

    nc.finalize()
    return nc


def get_nc():
    if "nc" not in _NC_CACHE:
        _NC_CACHE["nc"] = _build_nc()
    return _NC_CACHE["nc"]


def make_in_maps(cand, w, mask_np, shift):
    """Shard + lay out host inputs for the 8 cores."""
    import ml_dtypes

    f8 = ml_dtypes.float8_e4m3

    wf = w.reshape(E)
    wh = wf.astype(f8)
    wl = (wf - wh.astype(np.float32)).astype(f8)
    w_f8 = np.stack([wh, wl], axis=1)  # [128, 2] fp8
    prefix = np.zeros((128, PREFIX), np.uint8)
    prefix[:, 0:2] = np.ascontiguousarray(w_f8).view(np.uint8)

    # masked/padded candidate columns become this vector: score ~ -2000
    # (well below any real score; exp(s - shift) underflows to exact 0;
    # fits the fp8 e4m3 range)
    mvec = (float(MASKSCORE) / float(wf @ wf)) * wf  # [128] f32

    negshift = np.full((128, 1), -shift, np.float32)

    candT = cand.T  # [128, N] view
    in_maps = []
    for c in range(NCORES):
        xc = np.empty((128, PAD), np.float32)
        xc[:, :SHARD] = candT[:, c * SHARD : (c + 1) * SHARD]
        mflat = np.zeros(PAD, bool)
        mflat[:SHARD] = mask_np[0, c * SHARD : (c + 1) * SHARD] != 0
        xc[:, ~mflat] = mvec[:, None]
        hi_u8 = xc.astype(f8).view(np.uint8)  # [128, PAD]
        candB = np.empty((128, TOTB), np.uint8)
        candB[:, 0:PREFIX] = prefix
        candB[:, PREFIX:] = hi_u8
        in_maps.append({"candB": candB, "negshift": negshift})
    return in_maps


def _run_spmd(nc, in_maps):
    """run_bass_kernel_spmd with the optional NTFF-trace path made safe.

    If BASS_TRACE is set in the environment, run_bass_kernel_spmd needs the
    axon NTFF hook (antenv.axon_hooks) and an artifact upload; neither is
    guaranteed on this image.  Register the hook from the boot shim when
    missing, keep artifact upload local, and fall back to an untraced run
    on any trace-infrastructure failure.
    """
    import sys
    import types

    import concourse.bass_utils as bu

    try:
        import antenv.axon_hooks  # noqa: F401
    except ImportError:
        try:
            from trn_agent_boot.trn_boot import _ntff_profile_via_ctypes

            hook = _ntff_profile_via_ctypes("/opt/axon/libaxon_pjrt.so")
            mod = types.ModuleType("antenv.axon_hooks")
            mod.get_axon_ntff_profile_hook = lambda: hook
            sys.modules["antenv.axon_hooks"] = mod
        except Exception:
            os.environ["BASS_NEVER_TRACE"] = "1"

    if not getattr(bu.upload_artifacts, "_safe", False):
        orig_upload = bu.upload_artifacts

        def _safe_upload(tmpdir):
            try:
                return orig_upload(tmpdir)
            except Exception:
                return tmpdir

        _safe_upload._safe = True
        bu.upload_artifacts = _safe_upload

    try:
        return bu.run_bass_kernel_spmd(nc, in_maps, list(range(NCORES)))
    except Exception:
        if os.environ.get("BASS_NEVER_TRACE"):
            raise
        os.environ["BASS_NEVER_TRACE"] = "1"
        return bu.run_bass_kernel_spmd(nc, in_maps, list(range(NCORES)))


def kernel(current_node_emb, context_emb, candidate_node_embs, Wq, Wk, mask):
    global LAST_RESULTS, LAST_SCORES

    cur = np.asarray(current_node_emb, np.float32)
    ctxe = np.asarray(context_emb, np.float32)
    cand = np.ascontiguousarray(np.asarray(candidate_node_embs, np.float32))
    Wq_np = np.asarray(Wq, np.float32)
    Wk_np = np.asarray(Wk, np.float32)
    mask_np = np.asarray(mask)

    # tiny query projection; scores = w @ cand.T with w = (combined @ Wq.T) @ Wk
    combined = np.concatenate([cur, ctxe], axis=1)  # [1, 2E]
    query = (combined @ Wq_np.T).astype(np.float32)  # [1, H]
    w = (query @ Wk_np).astype(np.float32)  # [1, E]

    # fixed exp shift: safe upper bound on any score
    shift = float(max(40.0, 16.0 * np.linalg.norm(w)))

    in_maps = make_in_maps(cand, w, mask_np, shift)
    nc = get_nc()
    res = _run_spmd(nc, in_maps)
    LAST_RESULTS = res

    # ---- gather / merge ----
    all_scores = np.empty(N_TOTAL, np.float32)
    rowsums = np.empty((NCORES, 128), np.float64)
    for c in range(NCORES):
        ob = np.asarray(res.results[c]["outbuf"])  # [128, 200]
        all_scores[c * SHARD : (c + 1) * SHARD] = ob[:, :NSLICE].T.reshape(-1)[:SHARD]
        rowsums[c] = ob[:, NSLICE:].astype(np.float64).sum(axis=1)
    LAST_SCORES = all_scores

    # top-RESCORE candidates by device score; re-score them exactly on the
    # reference fp32 path (keys = cand @ Wk.T, s = query @ keys.T)
    sel = np.argpartition(all_scores, N_TOTAL - RESCORE)[N_TOTAL - RESCORE :]
    keys_sel = (cand[sel] @ Wk_np.T).astype(np.float32)  # [R, H]
    s_sel = (query @ keys_sel.T).astype(np.float32)[0]  # [R]

    # merge softmax statistics (the "all-reduce" step, done at gather time)
    m = np.float32(s_sel.max())
    Z = np.float32(np.exp(np.float64(shift) - np.float64(m)) * rowsums.sum())

    # exact probabilities of the re-scored candidates; top-50 threshold in
    # probability space, exactly like the reference
    p_sel = (np.exp(s_sel - m) / Z).astype(np.float32)
    th = np.sort(p_sel)[-TOPK]
    keep = p_sel >= th
    p_top = p_sel * keep
    S = p_top.sum(dtype=np.float32)
    fil_top = (p_top / (S + np.float32(1e-10))).astype(np.float32)

    log_probs_all = np.full((1, N_TOTAL), np.log(np.float32(1e-10)), np.float32)
    logits = np.full((1, N_TOTAL), -np.inf, np.float32)
    sel_keep = sel[keep]
    fil_keep = fil_top[keep]
    log_probs_all[0, sel_keep] = np.log(fil_keep + np.float32(1e-10))
    logits[0, sel_keep] = np.log(fil_keep)

    # categorical sample with jax key 42 (on host CPU, exact reference RNG)
    import jax

    cpu = jax.devices("cpu")[0]
    with jax.default_device(cpu):
        action_idx = np.asarray(
            jax.random.categorical(
                jax.random.key(42), jax.numpy.asarray(logits), axis=1
            )
        )
    log_prob_action = np.take_along_axis(logits, action_idx[:, None], axis=1)[:, 0]

    return log_probs_all, log_prob_action, action_idx
